# revision 17
# baseline (speedup 1.0000x reference)
"""Trainium2 kernel for nn_ContrastiveLoss (N=4096, D=1024), SPMD over 8 NeuronCores.

Strategy (row-sharded similarity matrix, fp8 DoubleRow matmuls):
  - Host (f64, O(N*D)): l2-normalize the four feature tensors, diag sims,
    pre_cos alignment term; scale back_* by 16 and quantize to e4m3 in
    DoubleRow-blocked layouts.
  - Each core (the O(N^2*D) part): its [512, 4096] slab of E = exp(Vn@An^T):
      * TensorE: fine-grained FD=128 warmup (~95ns/MM cold) sized to end
        right when the second first-pair DMA piece lands (~12.4us) - the
        HAM clock gate opens mid-warmup (~10.3us) and the real stream
        then runs warm with sub-us supply gaps (no MID-window demotion).
      * Input DMA on the two HWDGE rings ONLY, consumption-ordered; the
        8-lane DMAHW completion-sem pool naturally paces the later pieces:
          sync:   vn-m0, an0-lo, an0-hi, vn-m2, an2, an4, an6
          scalar: an1-lo, an1-hi, vn-m1, vn-m3, an3, an5, an7
        (Early SDMA service is slow, ~120-250GB/s until ~13us; the
        first-pair pieces land ~10.3/12.5/13.2/13.9us and the first-group
        MM order tracks that arrival.)
      * ScalarE: exp(PSUM/256) -> bf16 with fused f32 row-sum accumulator
        for all groups except the last three tail groups.
      * VectorE: bf16 column-sum partition-partial adds.
      * Tail (pair 3): group (3,1)'s adds complete the efold partial
        (m0+m1) which ships early; group (3,2)'s exp tile ships RAW
        (bf16) and group (3,3) splits into two PSUM tiles whose fp8 exps
        ship on sync+scalar concurrently - the host does those three
        groups' row sums, so the device tail is just exp->DMA with no
        accumulator-read chain, and the back-to-back tail DMAs keep the
        SDMA crew from idling (re-ramp) before the final transfers.
  - Host: O(N) final assembly in f64, including the 128-way column fold.

Measured (8 cores, core-0 profile): rel err ~3e-5.
"""

import os
import sys

import numpy as np

for _p in ("/opt/trn_rl_repo",):
    if _p not in sys.path and os.path.isdir(_p):
        sys.path.insert(0, _p)

N = 4096
D = 1024
NCORES = 8
ROWS = N // NCORES       # 512 rows per core
MCH = ROWS // 128        # 4 row chunks per core
KCH = D // 128           # 8 contraction chunks
NB = 512                 # matmul moving free dim
NCH = N // NB            # 8 column blocks

MARGIN = 0.2
BALANCE = 0.5
BIAS = 1.0
EPS = 1e-18

KD2 = KCH // 2   # fp8 DoubleRow: contraction chunks of 256 (2 x 128 rows)
FP8_SCALE = 16.0  # host pre-scale so e4m3 keeps the values out of subnormals

# FD=128 warmup matmuls bridging PE-barrier-exit (~7.45us) to the arrival
# of the second first-pair DMA piece (~12.4us): ~95ns each cold, ~70ns
# once the HAM gate opens mid-warmup.
WARM_MM = 54

_CACHE = {}
LAST_RESULT = None  # BassKernelResults of the most recent run (for test harness)


def _build_nc():
    import concourse.bass as bass  # noqa: F401
    import concourse.bacc as bacc
    import concourse.tile as tile
    from concourse import mybir
    from contextlib import ExitStack

    BF16 = mybir.dt.bfloat16
    F32 = mybir.dt.float32
    Exp = mybir.ActivationFunctionType.Exp

    NP2 = NCH // 2  # column-block pairs; each full exp covers 1024 cols

    nc = bacc.Bacc("TRN2", debug=False, num_devices=NCORES)

    FP8 = mybir.dt.float8e4
    DoubleRow = mybir.MatmulPerfMode.DoubleRow

    # DRAM I/O (per core). Layouts chosen so every DMA is one contiguous
    # [128, X] block.
    # m-major so the m=0 weight chunk can land first in its own small DMA:
    # vnT[p, m*KCH*128 + k2*256 + i*128 + r] = Vn_slab[m*128+r, (2*k2+i)*128+p]
    vnT_d = nc.dram_tensor("vnT", [128, KCH * ROWS], FP8, kind="ExternalInput")
    # anT[n, p, k2*2*NB + i*NB + c] = An[n*NB + c, (2*k2+i)*128 + p] * FP8_SCALE
    anT_d = nc.dram_tensor("anT", [NCH, 128, KCH * NB], FP8, kind="ExternalInput")

    # rowsum[p, np2*MCH + m] for all groups EXCEPT (3,2) and (3,3) - the
    # host recovers those two from the shipped exp tiles.
    rowsum_d = nc.dram_tensor("rowsum", [128, NP2 * MCH - 2], F32, kind="ExternalOutput")
    # efold[p, j]: sum over m-chunks of E[m*128+p, j] (m 0..3 for column
    # pairs 0..2; m 0..1 only for the last pair).
    efold_d = nc.dram_tensor("efold", [128, N], BF16, kind="ExternalOutput")
    # et32[p, j]: E[2*128+p, 3072+j] - group (3,2)'s exp tile, raw bf16.
    et32_d = nc.dram_tensor("et32", [128, 2 * NB], BF16, kind="ExternalOutput")
    # etl[p, j]: E[3*128+p, 3072+j] - group (3,3)'s exp tile, fp8.
    etl_d = nc.dram_tensor("etl", [128, 2 * NB], FP8, kind="ExternalOutput")

    with tile.TileContext(nc) as tc:
        with ExitStack() as ctx:
            singles = ctx.enter_context(tc.tile_pool(name="singles", bufs=1))

            dummy = singles.tile([128, 128], BF16, tag="dummy")
            nc.gpsimd.memset(dummy[:], 0.0)

            psum = ctx.enter_context(tc.tile_pool(name="mm_psum", bufs=3, space="PSUM"))
            foldp = ctx.enter_context(tc.tile_pool(name="fold_psum", bufs=2, space="PSUM"))
            epool = ctx.enter_context(tc.tile_pool(name="etile", bufs=3))

            # HAM warmup: dense FD=128 matmul stream from PE barrier exit.
            wps = foldp.tile([128, NB], mybir.dt.float32, tag="fold")
            for i in range(WARM_MM):
                nc.tensor.matmul(
                    wps[:, :128], dummy[:], dummy[:],
                    start=(i == 0), stop=(i == WARM_MM - 1),
                )

            # Input DMAs: consumption-ordered on the two HWDGE rings.
            vn_sb = singles.tile([128, KCH * ROWS], FP8, tag="vn")
            an_sb = []
            for n in range(NCH):
                an_t = singles.tile([128, KCH * NB], FP8, tag=f"an{n}")
                an_sb.append(an_t)
            VM = KCH * 128   # 1024 cols per m-chunk of vn
            AC = 2 * NB      # 1024 cols per k2 chunk of an
            AH = 2 * AC      # an block k2-half: 2048 cols

            nc.sync.dma_start(vn_sb[:, :VM], vnT_d.ap()[:, :VM])
            nc.scalar.dma_start(an_sb[1][:], anT_d.ap()[1])
            nc.sync.dma_start(an_sb[0][:], anT_d.ap()[0])
            nc.scalar.dma_start(vn_sb[:, VM : 2 * VM], vnT_d.ap()[:, VM : 2 * VM])
            nc.sync.dma_start(vn_sb[:, 2 * VM : 3 * VM], vnT_d.ap()[:, 2 * VM : 3 * VM])
            nc.scalar.dma_start(vn_sb[:, 3 * VM :], vnT_d.ap()[:, 3 * VM :])
            nc.sync.dma_start(an_sb[2][:], anT_d.ap()[2])
            nc.scalar.dma_start(an_sb[3][:], anT_d.ap()[3])
            nc.sync.dma_start(an_sb[4][:], anT_d.ap()[4])
            nc.scalar.dma_start(an_sb[5][:], anT_d.ap()[5])
            nc.sync.dma_start(an_sb[6][:], anT_d.ap()[6])
            nc.scalar.dma_start(an_sb[7][:], anT_d.ap()[7])

            efold16 = singles.tile([128, N], BF16, tag="efold16")
            rs = singles.tile([128, NP2 * MCH - 2], F32, tag="rs")

            # Main similarity slab. Column-pair outer (np2), row-chunk inner:
            # each group accumulates 8 DoubleRow matmuls into a [128, 1024]
            # PSUM tile (2 banks), then one wide exp (bf16 out, fused f32
            # row-sum) drains it. Column sums accumulate in bf16 (2x DVE).
            descale = 1.0 / (FP8_SCALE * FP8_SCALE)
            for np2 in range(NP2):
                nlo, nhi = 2 * np2, 2 * np2 + 1
                last_pair = np2 == NP2 - 1
                sl = slice(np2 * 2 * NB, (np2 + 1) * 2 * NB)
                for m in range(MCH):
                    last_group = last_pair and m == MCH - 1
                    # the last group gets two separate PSUM tiles (PSUM
                    # deps are tile-granular, so the lo exp must not share
                    # a tile with the hi half's matmuls; only [:, :NB] of
                    # each is used then). Single callsite: the pool sizes
                    # its per-buffer footprint per allocation site.
                    group_ps = [
                        psum.tile(
                            [128, 2 * NB], mybir.dt.float32,
                            name="ps", tag="ps",
                        )
                        for _ in range(2 if last_group else 1)
                    ]
                    if last_group:
                        ps_lo, ps_hi = group_ps
                    else:
                        ps = group_ps[0]
                    if np2 == 0 and m == 0:
                        # an1 (scalar ring head) typically lands ~1us
                        # before an0 (behind vn-m0 on sync)
                        order = [(1, nhi, k2) for k2 in range(KD2)] + [
                            (0, nlo, k2) for k2 in range(KD2)
                        ]
                    elif last_group:
                        # lo half first so its exp + DMA overlap the hi MMs
                        order = [(0, nlo, k2) for k2 in range(KD2)] + [
                            (1, nhi, k2) for k2 in range(KD2)
                        ]
                    else:
                        order = [
                            (half, nn, k2)
                            for k2 in range(KD2)
                            for half, nn in ((0, nlo), (1, nhi))
                        ]
                    for half, nn, k2 in order:
                        w3 = (
                            vn_sb[:, m * VM + k2 * 256 : m * VM + (k2 + 1) * 256]
                            .rearrange("p (i c) -> p i c", i=2)
                        )
                        a3 = (
                            an_sb[nn][:, k2 * AC : (k2 + 1) * AC]
                            .rearrange("p (i c) -> p i c", i=2)
                        )
                        if last_group:
                            out_ap = (ps_lo if nn == nlo else ps_hi)[:, :NB]
                        else:
                            out_ap = ps[:, half * NB : (half + 1) * NB]
                        nc.tensor.matmul(
                            out_ap,
                            w3,
                            a3,
                            start=(k2 == 0),
                            stop=(k2 == KD2 - 1),
                            perf_mode=DoubleRow,
                        )
                    col = np2 * MCH + m
                    if last_group:
                        # tail: plain exps, raw fp8 ships on both rings;
                        # host does these row sums from the shipped tiles
                        et_lo = epool.tile([128, NB], FP8, tag="et_lo")
                        et_hi = epool.tile([128, NB], FP8, tag="et_hi")
                        nc.scalar.activation(et_lo[:], ps_lo[:, :NB], Exp, scale=descale)
                        nc.sync.dma_start(etl_d.ap()[:, :NB], et_lo[:])
                        nc.scalar.activation(et_hi[:], ps_hi[:, :NB], Exp, scale=descale)
                        nc.scalar.dma_start(etl_d.ap()[:, NB:], et_hi[:])
                    elif last_pair and m == MCH - 2:
                        # (3,2): raw bf16 tile ships; no accumulator, no
                        # efold adds - host folds it. Keeps ACT free for
                        # the tail exps and the SDMA crew warm.
                        et32 = epool.tile([128, 2 * NB], BF16, tag="et32")
                        nc.scalar.activation(et32[:], ps[:], Exp, scale=descale)
                        nc.sync.dma_start(et32_d.ap(), et32[:])
                    elif m == 0:
                        nc.scalar.activation(
                            efold16[:, sl], ps[:], Exp, scale=descale,
                            accum_out=rs[:, col : col + 1],
                        )
                    else:
                        et = epool.tile([128, 2 * NB], BF16)
                        nc.scalar.activation(
                            et[:], ps[:], Exp, scale=descale,
                            accum_out=rs[:, col : col + 1],
                        )
                        if m == MCH - 1:
                            # split in halves so each fold can start sooner
                            for h in range(2):
                                hs = slice(
                                    (np2 * 2 + h) * NB, (np2 * 2 + h + 1) * NB
                                )
                                nc.vector.tensor_add(
                                    efold16[:, hs], efold16[:, hs],
                                    et[:, h * NB : (h + 1) * NB],
                                )
                        else:
                            nc.vector.tensor_add(efold16[:, sl], efold16[:, sl], et[:])
                    if last_pair and m == 1:
                        # pair 3's efold partial is m0+m1 only (m2 and m3
                        # ship raw); final after m=1's add - ship it now
                        nc.sync.dma_start(efold_d.ap()[:, sl], efold16[:, sl])
                if not last_pair:
                    # ship this pair's finished partition-partial while the
                    # stream continues
                    nc.sync.dma_start(efold_d.ap()[:, sl], efold16[:, sl])

            # rowsum: ready once group (3,1)'s accumulator has been read
            nc.sync.dma_start(rowsum_d.ap(), rs[:])

    nc.compile()
    return nc


def _get_nc():
    if "nc" not in _CACHE:
        _CACHE["nc"] = _build_nc()
    return _CACHE["nc"]


def _prep_inputs(pre_VF, pre_AF, back_VF, back_AF):
    """Normalize + relayout on host; returns per-core in_maps + host scalars."""
    import ml_dtypes

    def l2n(x):
        x = np.asarray(x, dtype=np.float64)
        return x / np.sqrt((x * x).sum(-1, keepdims=True) + EPS)

    Vn = l2n(back_VF)
    An = l2n(back_AF)
    diag = np.einsum("ij,ij->i", Vn, An)  # f64, exact-ish
    L_pre = float(np.einsum("ij,ij->i", l2n(pre_VF), l2n(pre_AF)).sum())

    fp8 = ml_dtypes.float8_e4m3
    Vn8 = (Vn * FP8_SCALE).astype(fp8)
    An8 = (An * FP8_SCALE).astype(fp8)

    # anT[n, p, k2*2*NB + i*NB + c] = An8[n*NB + c, (2*k2+i)*128 + p]
    anT = np.ascontiguousarray(
        An8.reshape(NCH, NB, KD2, 2, 128)
        .transpose(0, 4, 2, 3, 1)
        .reshape(NCH, 128, KCH * NB)
    )

    in_maps = []
    for c in range(NCORES):
        sl = slice(c * ROWS, (c + 1) * ROWS)
        # vnT[p, m*KCH*128 + k2*256 + i*128 + r] = Vn8_slab[m*128+r, (2k2+i)*128+p]
        vnT = np.ascontiguousarray(
            Vn8[sl]
            .reshape(MCH, 128, KD2, 2, 128)
            .transpose(4, 0, 2, 3, 1)
            .reshape(128, KCH * ROWS)
        )
        in_maps.append({"vnT": vnT, "anT": anT})
    return in_maps, diag, L_pre


def _assemble(outs, diag, L_pre):
    """O(N) final reduction on host, f64."""
    NP2 = NCH // 2
    rowsum_chunks = []
    for c in range(NCORES):
        rs_c = outs[c]["rowsum"].astype(np.float64)   # [128, 14]
        et32 = outs[c]["et32"].astype(np.float64)     # [128, 1024] (3,2)
        etl = outs[c]["etl"].astype(np.float64)       # [128, 1024] (3,3)
        grid = np.empty((128, NP2, MCH), dtype=np.float64)
        grid.reshape(128, NP2 * MCH)[:, : NP2 * MCH - 2] = rs_c
        grid[:, NP2 - 1, MCH - 2] = et32.sum(axis=1)
        grid[:, NP2 - 1, MCH - 1] = etl.sum(axis=1)
        rowsum_chunks.append(grid.sum(1).T.reshape(ROWS))
    rowsum = np.concatenate(rowsum_chunks)
    colsum = np.zeros(N, dtype=np.float64)
    for c in range(NCORES):
        colsum += outs[c]["efold"].astype(np.float64).sum(axis=0)
        colsum[3 * N // 4 :] += outs[c]["et32"].astype(np.float64).sum(axis=0)
        colsum[3 * N // 4 :] += outs[c]["etl"].astype(np.float64).sum(axis=0)

    dE = np.exp(diag)
    pos = np.exp(diag - MARGIN)
    neg_V = rowsum - dE
    neg_A = colsum - dE
    L_V = np.log(pos / (pos + neg_V)).sum()
    L_A = np.log(pos / (pos + neg_A)).sum()

    loss = BALANCE * (-1.0 / BIAS) * (L_V + L_A) + (1.0 - BALANCE) * L_pre
    return np.array(loss, dtype=np.float32)


def kernel(pre_VF, pre_AF, back_VF, back_AF):
    global LAST_RESULT
    from concourse import bass_utils

    nc = _get_nc()
    in_maps, diag, L_pre = _prep_inputs(pre_VF, pre_AF, back_VF, back_AF)
    res = bass_utils.run_bass_kernel_spmd(nc, in_maps, core_ids=list(range(NCORES)))
    LAST_RESULT = res
    return _assemble(res.results, diag, L_pre)


# revision 18
# speedup vs baseline: 1.1751x; 1.1751x over previous
"""Trainium2 kernel for nn_ContrastiveLoss (N=4096, D=1024), SPMD over 8 NeuronCores.

Strategy (row-sharded similarity matrix, fp8 DoubleRow matmuls):
  - Host (f64, O(N*D)): l2-normalize the four feature tensors, diag sims,
    pre_cos alignment term; scale back_* by 16 and quantize to e4m3 in
    DoubleRow-blocked layouts.
  - Each core (the O(N^2*D) part): its [512, 4096] slab of E = exp(Vn@An^T):
      * TensorE: fine-grained FD=128 warmup (~95ns/MM cold) sized to end
        right when the second first-pair DMA piece lands (~12.4us) - the
        HAM clock gate opens mid-warmup (~10.3us) and the real stream
        then runs warm with sub-us supply gaps (no MID-window demotion).
      * Input DMA on the two HWDGE rings ONLY, consumption-ordered; the
        8-lane DMAHW completion-sem pool naturally paces the later pieces:
          sync:   vn-m0, an0-lo, an0-hi, vn-m2, an2, an4, an6
          scalar: an1-lo, an1-hi, vn-m1, vn-m3, an3, an5, an7
        (Early SDMA service is slow, ~120-250GB/s until ~13us; the
        first-pair pieces land ~10.3/12.5/13.2/13.9us and the first-group
        MM order tracks that arrival.)
      * ScalarE: exp(PSUM/256) -> bf16 with fused f32 row-sum accumulator
        for all groups except the last three tail groups.
      * VectorE: bf16 column-sum partition-partial adds.
      * Tail (pair 3): group (3,1)'s adds complete the efold partial
        (m0+m1) which ships early; group (3,2)'s exp tile ships RAW
        (bf16) and group (3,3) splits into two PSUM tiles whose fp8 exps
        ship on sync+scalar concurrently - the host does those three
        groups' row sums, so the device tail is just exp->DMA with no
        accumulator-read chain, and the back-to-back tail DMAs keep the
        SDMA crew from idling (re-ramp) before the final transfers.
  - Host: O(N) final assembly in f64, including the 128-way column fold.

Measured (8 cores, core-0 profile): rel err ~3e-5.
"""

import os
import sys

import numpy as np

for _p in ("/opt/trn_rl_repo",):
    if _p not in sys.path and os.path.isdir(_p):
        sys.path.insert(0, _p)

N = 4096
D = 1024
NCORES = 8
ROWS = N // NCORES       # 512 rows per core
MCH = ROWS // 128        # 4 row chunks per core
KCH = D // 128           # 8 contraction chunks
NB = 512                 # matmul moving free dim
NCH = N // NB            # 8 column blocks

MARGIN = 0.2
BALANCE = 0.5
BIAS = 1.0
EPS = 1e-18

KD2 = KCH // 2   # fp8 DoubleRow: contraction chunks of 256 (2 x 128 rows)
FP8_SCALE = 16.0  # host pre-scale so e4m3 keeps the values out of subnormals

# FD=128 warmup matmuls bridging PE-barrier-exit (~7.45us) to the arrival
# of the second first-pair DMA piece (~12.4us): ~95ns each cold, ~70ns
# once the HAM gate opens mid-warmup.
WARM_MM = 60

_CACHE = {}
LAST_RESULT = None  # BassKernelResults of the most recent run (for test harness)


def _build_nc():
    import concourse.bass as bass  # noqa: F401
    import concourse.bacc as bacc
    import concourse.tile as tile
    from concourse import mybir
    from contextlib import ExitStack

    BF16 = mybir.dt.bfloat16
    F32 = mybir.dt.float32
    Exp = mybir.ActivationFunctionType.Exp

    NP2 = NCH // 2  # column-block pairs; each full exp covers 1024 cols

    nc = bacc.Bacc("TRN2", debug=False, num_devices=NCORES)

    FP8 = mybir.dt.float8e4
    DoubleRow = mybir.MatmulPerfMode.DoubleRow

    # DRAM I/O (per core). Layouts chosen so every DMA is one contiguous
    # [128, X] block.
    # m-major so the m=0 weight chunk can land first in its own small DMA:
    # vnT[p, m*KCH*128 + k2*256 + i*128 + r] = Vn_slab[m*128+r, (2*k2+i)*128+p]
    vnT_d = nc.dram_tensor("vnT", [128, KCH * ROWS], FP8, kind="ExternalInput")
    # anT[n, p, k2*2*NB + i*NB + c] = An[n*NB + c, (2*k2+i)*128 + p] * FP8_SCALE
    anT_d = nc.dram_tensor("anT", [NCH, 128, KCH * NB], FP8, kind="ExternalInput")

    # rowsum[p, np2*MCH + m] for all groups EXCEPT (3,2) and (3,3) - the
    # host recovers those two from the shipped exp tiles.
    rowsum_d = nc.dram_tensor("rowsum", [128, NP2 * MCH - 2], F32, kind="ExternalOutput")
    # efold[p, j]: sum over m-chunks of E[m*128+p, j] (m 0..3 for column
    # pairs 0..2; m 0..1 only for the last pair).
    efold_d = nc.dram_tensor("efold", [128, N], BF16, kind="ExternalOutput")
    # et32[p, j]: E[2*128+p, 3072+j] - group (3,2)'s exp tile, raw bf16.
    et32_d = nc.dram_tensor("et32", [128, 2 * NB], BF16, kind="ExternalOutput")
    # etl[p, j]: E[3*128+p, 3072+j] - group (3,3)'s exp tile, fp8.
    etl_d = nc.dram_tensor("etl", [128, 2 * NB], FP8, kind="ExternalOutput")

    with tile.TileContext(nc) as tc:
        with ExitStack() as ctx:
            singles = ctx.enter_context(tc.tile_pool(name="singles", bufs=1))

            dummy = singles.tile([128, 128], BF16, tag="dummy")
            nc.gpsimd.memset(dummy[:], 0.0)

            psum = ctx.enter_context(tc.tile_pool(name="mm_psum", bufs=3, space="PSUM"))
            foldp = ctx.enter_context(tc.tile_pool(name="fold_psum", bufs=2, space="PSUM"))
            epool = ctx.enter_context(tc.tile_pool(name="etile", bufs=3))

            # HAM warmup: dense FD=128 matmul stream from PE barrier exit.
            wps = foldp.tile([128, NB], mybir.dt.float32, tag="fold")
            for i in range(WARM_MM):
                nc.tensor.matmul(
                    wps[:, :128], dummy[:], dummy[:],
                    start=(i == 0), stop=(i == WARM_MM - 1),
                )

            # Input DMAs: consumption-ordered on the two HWDGE rings.
            vn_sb = singles.tile([128, KCH * ROWS], FP8, tag="vn")
            an_sb = []
            for n in range(NCH):
                an_t = singles.tile([128, KCH * NB], FP8, tag=f"an{n}")
                an_sb.append(an_t)
            VM = KCH * 128   # 1024 cols per m-chunk of vn
            AC = 2 * NB      # 1024 cols per k2 chunk of an
            AH = 2 * AC      # an block k2-half: 2048 cols

            nc.sync.dma_start(vn_sb[:, :VM], vnT_d.ap()[:, :VM])
            nc.scalar.dma_start(an_sb[1][:, :AH], anT_d.ap()[1][:, :AH])
            nc.sync.dma_start(an_sb[0][:, :AH], anT_d.ap()[0][:, :AH])
            nc.scalar.dma_start(an_sb[1][:, AH:], anT_d.ap()[1][:, AH:])
            nc.sync.dma_start(an_sb[0][:, AH:], anT_d.ap()[0][:, AH:])
            nc.scalar.dma_start(vn_sb[:, VM : 2 * VM], vnT_d.ap()[:, VM : 2 * VM])
            nc.sync.dma_start(vn_sb[:, 2 * VM : 3 * VM], vnT_d.ap()[:, 2 * VM : 3 * VM])
            nc.scalar.dma_start(vn_sb[:, 3 * VM :], vnT_d.ap()[:, 3 * VM :])
            nc.sync.dma_start(an_sb[2][:], anT_d.ap()[2])
            nc.scalar.dma_start(an_sb[3][:], anT_d.ap()[3])
            nc.sync.dma_start(an_sb[4][:], anT_d.ap()[4])
            nc.scalar.dma_start(an_sb[5][:], anT_d.ap()[5])
            nc.sync.dma_start(an_sb[6][:], anT_d.ap()[6])
            nc.scalar.dma_start(an_sb[7][:], anT_d.ap()[7])

            efold16 = singles.tile([128, N], BF16, tag="efold16")
            rs = singles.tile([128, NP2 * MCH - 2], F32, tag="rs")

            # Main similarity slab. Column-pair outer (np2), row-chunk inner:
            # each group accumulates 8 DoubleRow matmuls into a [128, 1024]
            # PSUM tile (2 banks), then one wide exp (bf16 out, fused f32
            # row-sum) drains it. Column sums accumulate in bf16 (2x DVE).
            descale = 1.0 / (FP8_SCALE * FP8_SCALE)
            for np2 in range(NP2):
                nlo, nhi = 2 * np2, 2 * np2 + 1
                last_pair = np2 == NP2 - 1
                sl = slice(np2 * 2 * NB, (np2 + 1) * 2 * NB)
                for m in range(MCH):
                    last_group = last_pair and m == MCH - 1
                    # the last group gets two separate PSUM tiles (PSUM
                    # deps are tile-granular, so the lo exp must not share
                    # a tile with the hi half's matmuls; only [:, :NB] of
                    # each is used then). Single callsite: the pool sizes
                    # its per-buffer footprint per allocation site.
                    group_ps = [
                        psum.tile(
                            [128, 2 * NB], mybir.dt.float32,
                            name="ps", tag="ps",
                        )
                        for _ in range(2 if last_group else 1)
                    ]
                    if last_group:
                        ps_lo, ps_hi = group_ps
                    else:
                        ps = group_ps[0]
                    if np2 == 0 and m == 0:
                        # matches piece arrival: an1-lo, an0-lo, an1-hi, an0-hi
                        order = [
                            (1, nhi, 0), (1, nhi, 1), (0, nlo, 0), (0, nlo, 1),
                            (1, nhi, 2), (1, nhi, 3), (0, nlo, 2), (0, nlo, 3),
                        ]
                    elif last_group:
                        # lo half first so its exp + DMA overlap the hi MMs
                        order = [(0, nlo, k2) for k2 in range(KD2)] + [
                            (1, nhi, k2) for k2 in range(KD2)
                        ]
                    else:
                        order = [
                            (half, nn, k2)
                            for k2 in range(KD2)
                            for half, nn in ((0, nlo), (1, nhi))
                        ]
                    for half, nn, k2 in order:
                        w3 = (
                            vn_sb[:, m * VM + k2 * 256 : m * VM + (k2 + 1) * 256]
                            .rearrange("p (i c) -> p i c", i=2)
                        )
                        a3 = (
                            an_sb[nn][:, k2 * AC : (k2 + 1) * AC]
                            .rearrange("p (i c) -> p i c", i=2)
                        )
                        if last_group:
                            out_ap = (ps_lo if nn == nlo else ps_hi)[:, :NB]
                        else:
                            out_ap = ps[:, half * NB : (half + 1) * NB]
                        nc.tensor.matmul(
                            out_ap,
                            w3,
                            a3,
                            start=(k2 == 0),
                            stop=(k2 == KD2 - 1),
                            perf_mode=DoubleRow,
                        )
                    col = np2 * MCH + m
                    if last_group:
                        # tail: plain exps, raw fp8 ships on both rings;
                        # host does these row sums from the shipped tiles
                        et_lo = epool.tile([128, NB], FP8, tag="et_lo")
                        et_hi = epool.tile([128, NB], FP8, tag="et_hi")
                        nc.scalar.activation(et_lo[:], ps_lo[:, :NB], Exp, scale=descale)
                        nc.sync.dma_start(etl_d.ap()[:, :NB], et_lo[:])
                        nc.scalar.activation(et_hi[:], ps_hi[:, :NB], Exp, scale=descale)
                        nc.scalar.dma_start(etl_d.ap()[:, NB:], et_hi[:])
                    elif last_pair and m == MCH - 2:
                        # (3,2): raw bf16 tile ships; no accumulator, no
                        # efold adds - host folds it. Keeps ACT free for
                        # the tail exps and the SDMA crew warm.
                        et32 = epool.tile([128, 2 * NB], BF16, tag="et32")
                        nc.scalar.activation(et32[:], ps[:], Exp, scale=descale)
                        nc.sync.dma_start(et32_d.ap(), et32[:])
                    elif m == 0:
                        nc.scalar.activation(
                            efold16[:, sl], ps[:], Exp, scale=descale,
                            accum_out=rs[:, col : col + 1],
                        )
                    else:
                        et = epool.tile([128, 2 * NB], BF16)
                        nc.scalar.activation(
                            et[:], ps[:], Exp, scale=descale,
                            accum_out=rs[:, col : col + 1],
                        )
                        if m == MCH - 1:
                            # split in halves so each fold can start sooner
                            for h in range(2):
                                hs = slice(
                                    (np2 * 2 + h) * NB, (np2 * 2 + h + 1) * NB
                                )
                                nc.vector.tensor_add(
                                    efold16[:, hs], efold16[:, hs],
                                    et[:, h * NB : (h + 1) * NB],
                                )
                        else:
                            nc.vector.tensor_add(efold16[:, sl], efold16[:, sl], et[:])
                    if last_pair and m == 1:
                        # pair 3's efold partial is m0+m1 only (m2 and m3
                        # ship raw); final after m=1's add - ship it now
                        nc.sync.dma_start(efold_d.ap()[:, sl], efold16[:, sl])
                if not last_pair:
                    # ship this pair's finished partition-partial while the
                    # stream continues
                    nc.sync.dma_start(efold_d.ap()[:, sl], efold16[:, sl])

            # rowsum: ready once group (3,1)'s accumulator has been read
            nc.sync.dma_start(rowsum_d.ap(), rs[:])

    nc.compile()
    return nc


def _get_nc():
    if "nc" not in _CACHE:
        _CACHE["nc"] = _build_nc()
    return _CACHE["nc"]


def _prep_inputs(pre_VF, pre_AF, back_VF, back_AF):
    """Normalize + relayout on host; returns per-core in_maps + host scalars."""
    import ml_dtypes

    def l2n(x):
        x = np.asarray(x, dtype=np.float64)
        return x / np.sqrt((x * x).sum(-1, keepdims=True) + EPS)

    Vn = l2n(back_VF)
    An = l2n(back_AF)
    diag = np.einsum("ij,ij->i", Vn, An)  # f64, exact-ish
    L_pre = float(np.einsum("ij,ij->i", l2n(pre_VF), l2n(pre_AF)).sum())

    fp8 = ml_dtypes.float8_e4m3
    Vn8 = (Vn * FP8_SCALE).astype(fp8)
    An8 = (An * FP8_SCALE).astype(fp8)

    # anT[n, p, k2*2*NB + i*NB + c] = An8[n*NB + c, (2*k2+i)*128 + p]
    anT = np.ascontiguousarray(
        An8.reshape(NCH, NB, KD2, 2, 128)
        .transpose(0, 4, 2, 3, 1)
        .reshape(NCH, 128, KCH * NB)
    )

    in_maps = []
    for c in range(NCORES):
        sl = slice(c * ROWS, (c + 1) * ROWS)
        # vnT[p, m*KCH*128 + k2*256 + i*128 + r] = Vn8_slab[m*128+r, (2k2+i)*128+p]
        vnT = np.ascontiguousarray(
            Vn8[sl]
            .reshape(MCH, 128, KD2, 2, 128)
            .transpose(4, 0, 2, 3, 1)
            .reshape(128, KCH * ROWS)
        )
        in_maps.append({"vnT": vnT, "anT": anT})
    return in_maps, diag, L_pre


def _assemble(outs, diag, L_pre):
    """O(N) final reduction on host, f64."""
    NP2 = NCH // 2
    rowsum_chunks = []
    for c in range(NCORES):
        rs_c = outs[c]["rowsum"].astype(np.float64)   # [128, 14]
        et32 = outs[c]["et32"].astype(np.float64)     # [128, 1024] (3,2)
        etl = outs[c]["etl"].astype(np.float64)       # [128, 1024] (3,3)
        grid = np.empty((128, NP2, MCH), dtype=np.float64)
        grid.reshape(128, NP2 * MCH)[:, : NP2 * MCH - 2] = rs_c
        grid[:, NP2 - 1, MCH - 2] = et32.sum(axis=1)
        grid[:, NP2 - 1, MCH - 1] = etl.sum(axis=1)
        rowsum_chunks.append(grid.sum(1).T.reshape(ROWS))
    rowsum = np.concatenate(rowsum_chunks)
    colsum = np.zeros(N, dtype=np.float64)
    for c in range(NCORES):
        colsum += outs[c]["efold"].astype(np.float64).sum(axis=0)
        colsum[3 * N // 4 :] += outs[c]["et32"].astype(np.float64).sum(axis=0)
        colsum[3 * N // 4 :] += outs[c]["etl"].astype(np.float64).sum(axis=0)

    dE = np.exp(diag)
    pos = np.exp(diag - MARGIN)
    neg_V = rowsum - dE
    neg_A = colsum - dE
    L_V = np.log(pos / (pos + neg_V)).sum()
    L_A = np.log(pos / (pos + neg_A)).sum()

    loss = BALANCE * (-1.0 / BIAS) * (L_V + L_A) + (1.0 - BALANCE) * L_pre
    return np.array(loss, dtype=np.float32)


def kernel(pre_VF, pre_AF, back_VF, back_AF):
    global LAST_RESULT
    from concourse import bass_utils

    nc = _get_nc()
    in_maps, diag, L_pre = _prep_inputs(pre_VF, pre_AF, back_VF, back_AF)
    res = bass_utils.run_bass_kernel_spmd(nc, in_maps, core_ids=list(range(NCORES)))
    LAST_RESULT = res
    return _assemble(res.results, diag, L_pre)
